# revision 31
# baseline (speedup 1.0000x reference)
"""Trainium2 Bass kernel for ContactSignedDistanceHead.

Computes, for B=16 batches of K=256 particles with D=128 features:
    hi = particles @ W1[:D]; hj = particles @ W1[D:]
    sd[b,i,j] = sum_h W2[h] * gelu(hi[b,i,h] + hj[b,j,h] + b1[h]) + b2
    loss      = mean((sd - pairwise_dist(positions))**2)

Sharding: data-parallel over batch, 2 batches per core across 8 cores.
The [B,K,K,H] intermediate (268 MB) is never materialized to HBM: each
core streams it through SBUF in [128, 32*256] chunks.

Device layout (per batch):
  - partitions p = (h, i2): p < 64 -> (h=p, i2=0), p >= 64 -> (h=p-64, i2=1)
    where i = i2*128 + i_sub.
  - hib2 [128, 128]: hib2[(h,i2), i_sub] = hi[i2*128+i_sub, h]
  - hjb2 [128, 256]: hjb2[(h,i2), j]     = hj[j, h] + b1[h]   (same for both i2)
  - X[(h,i2), (i_sub, j)] = hib2 bcast over j + hjb2 bcast over i_sub (DVE)
  - G = gelu(X)                                                 (ACT, exact)
  - sd rows: matmul with block-diag W2 lhsT [128, 2]            (PE, contract h)
    -> psum [2, N], DMA'd straight to HBM.
"""

import numpy as np

import concourse.bass as bass
import concourse.mybir as mybir
from concourse.bass_utils import run_bass_kernel_spmd
from concourse.tile import TileContext

N_CORES = 8
B, K, D, H = 16, 256, 128, 64
BPC = B // N_CORES  # batches per core

F32 = mybir.dt.float32
F16 = mybir.dt.float16
ADD = mybir.AluOpType.add

# i_sub chunk size for the streamed X/G tiles: [128, CH*256] per chunk
CH = 16
NCHUNK = 128 // CH
# which global chunk indices (b*NCHUNK+c) run their broadcast-add on GPSIMD
GP_CHUNKS = frozenset((1, 3, 5, 7, 9, 11, 13))
# fraction of evacuations on DVE: t % EVAC_MOD != 0 -> DVE
EVAC_MOD = 0

# Results of the last device run (exec_time_ns etc.), for test harnesses.
LAST_RESULTS = None
_CACHED_NC = None


def _split_multi_waits(nc):
    """The walrus codegen in this container accepts at most ONE sync-wait
    per lowered compute instruction (setupSyncWait: 'Too many sync wait
    commands'). Tile emits several. Hoist all but one wait of every such
    instruction onto standalone same-engine EventSemaphore instructions
    placed immediately before it — semantically identical (the engine
    blocks on each wait in turn instead of on all at once)."""
    skip = {"EventSemaphore", "AllEngineBarrier"}
    cnt = 0
    for fn in nc.m.functions:
        for blk in fn.blocks:
            out = []
            changed = False
            for inst in blk.instructions:
                si = inst.sync_info
                if si is not None and len(si.on_wait) > 1 and inst.opcode not in skip:
                    waits = list(si.on_wait)
                    for w in waits[:-1]:
                        cnt += 1
                        ev = mybir.InstEventSemaphore(
                            name=f"I-waitsplit-{cnt}",
                            ins=[],
                            outs=[],
                            sync_info=mybir.SyncInfo(on_wait=[w], on_update=[]),
                        )
                        ev.engine = inst.engine
                        out.append(ev)
                    inst.sync_info = mybir.SyncInfo(
                        on_wait=[waits[-1]], on_update=list(si.on_update)
                    )
                    changed = True
                out.append(inst)
            if changed:
                blk.instructions = out
    return cnt


def _build_bass(split_waits=True, psum_init=False):
    nc = bass.Bass("TRN2")

    # particles pre-transposed on host to [b, d, k] so the contract dim (d)
    # lands on partitions with a plain DMA (no PE transpose needed)
    p_in = nc.dram_tensor("particlesT", [BPC, D, K], F16, kind="ExternalInput")
    w1_in = nc.dram_tensor("W1", [2 * D, H], F16, kind="ExternalInput")
    b1_in = nc.dram_tensor("b1", [H], F32, kind="ExternalInput")
    w2_in = nc.dram_tensor("W2", [H, 1], F32, kind="ExternalInput")
    sd_out = nc.dram_tensor("sd", [BPC, K, K], F32, kind="ExternalOutput")

    gelu = mybir.ActivationFunctionType.Gelu

    with TileContext(nc) as tc:
        with (
            tc.tile_pool(name="consts", bufs=1) as cpool,
            tc.tile_pool(name="setup", bufs=2) as spool,
            tc.tile_pool(name="xg", bufs=4) as xgpool,
            tc.tile_pool(name="pss", bufs=1, space="PSUM") as pss,
            tc.tile_pool(name="psd", bufs=3, space="PSUM") as psd,
        ):
            # ---- inputs; batch-0 particles first so PE starts ASAP ----
            pT0 = spool.tile([128, K], F16, tag="pT")
            nc.sync.dma_start(pT0[:], p_in[0])
            # W1 halves as matmul stationaries [d=128, h=64]
            w1a = cpool.tile([128, H], F16)
            w1b = cpool.tile([128, H], F16)
            nc.sync.dma_start(w1a[:], w1_in[0:D, :])
            nc.sync.dma_start(w1b[:], w1_in[D:, :])

            # block-diagonal W2: rows 0:64 col 0 = W2, rows 64:128 col 1 = W2
            # fp16 so the reduce matmuls run at full PE rate (fp32 is 1/4).
            # (cast copies must be partition-aligned: engines cannot move
            # data across partitions, only DMA can.)
            w2stage = cpool.tile([128, 1], F32)
            nc.scalar.dma_start(w2stage[0:64, :], w2_in[:, :])
            nc.scalar.dma_start(w2stage[64:128, :], w2_in[:, :])
            w2blk = cpool.tile([128, 2], F16)
            nc.vector.memset(w2blk[:], 0.0)
            nc.vector.tensor_copy(w2blk[0:64, 0:1], w2stage[0:64, :])
            nc.vector.tensor_copy(w2blk[64:128, 1:2], w2stage[64:128, :])

            # b1 stacked twice along partitions -> [128, 1]
            b1t = cpool.tile([128, 1], F32)
            nc.scalar.dma_start(b1t[0:64, :], b1_in[:].unsqueeze(1))
            nc.scalar.dma_start(b1t[64:128, :], b1_in[:].unsqueeze(1))

            pT1 = spool.tile([128, K], F16, tag="pT")
            nc.sync.dma_start(pT1[:], p_in[1])
            pT_tiles = [pT0, pT1]

            ev_idx = 0
            for b in range(BPC):
                # ---- per-batch setup ----
                pT = pT_tiles[b]

                # hi and hj share one psum bank: [:, 0:128] = hi, [:, 128:384] = hj
                hij_ps = pss.tile([128, 384], F32, tag="hij", bufs=2)
                nc.tensor.matmul(
                    hij_ps[0:64, 0:128], w1a[:], pT[:, 0:128], start=True, stop=True
                )
                nc.tensor.matmul(
                    hij_ps[64:128, 0:128],
                    w1a[:],
                    pT[:, 128:256],
                    start=True,
                    stop=True,
                    tile_position=(0, 64),
                )
                nc.tensor.matmul(
                    hij_ps[0:64, 128:384], w1b[:], pT[:], start=True, stop=True
                )
                nc.tensor.matmul(
                    hij_ps[64:128, 128:384],
                    w1b[:],
                    pT[:],
                    start=True,
                    stop=True,
                    tile_position=(0, 64),
                )

                hib2 = spool.tile([128, 128], F32, tag="hib2")
                nc.vector.tensor_copy(hib2[:], hij_ps[:, 0:128])
                hjb2 = spool.tile([128, K], F32, tag="hjb2")
                nc.vector.tensor_scalar(hjb2[:], hij_ps[:, 128:384], b1t[:], None, ADD)

                # ---- streamed main loop ----
                for c in range(NCHUNK):
                    x = xgpool.tile([128, CH, K], F32, tag="X")
                    addeng = nc.gpsimd if (b * NCHUNK + c) in GP_CHUNKS else nc.vector
                    addeng.tensor_tensor(
                        x[:],
                        hjb2[:].unsqueeze(1).to_broadcast((128, CH, K)),
                        hib2[:, c * CH : (c + 1) * CH]
                        .unsqueeze(2)
                        .to_broadcast((128, CH, K)),
                        ADD,
                    )
                    g = xgpool.tile([128, CH, K], F16, tag="G")
                    nc.scalar.activation(g[:], x[:], gelu)
                    gf = g[:].rearrange("p a b -> p (a b)")

                    # reduce over h via PE: block-diag W2 lhsT gives out rows
                    # (i2=0, i2=1). Four tile_position column-slots pack the
                    # outputs at partition bases 0/32/64/96 of the same psum
                    # banks, so evacuation runs at full free-dim width.
                    for t in range(CH // 16):
                        sdp = psd.tile([128, 1024], F32, tag="sd")
                        if psum_init:  # sim-only: satisfy uninit-read check
                            nc.vector.memset(sdp[:], 0.0)
                        for s in range(8):
                            kk, q = s // 2, s % 2
                            col0 = (16 * t + 2 * s) * K
                            nc.tensor.matmul(
                                sdp[32 * kk : 32 * kk + 2, 512 * q : 512 * (q + 1)],
                                w2blk[:],
                                gf[:, col0 : col0 + 512],
                                start=True,
                                stop=True,
                                tile_position=(0, 32 * kk),
                            )
                        ev = spool.tile([128, 1024], F32, tag="ev")
                        if EVAC_MOD and ev_idx % EVAC_MOD == 0:
                            nc.scalar.copy(ev[:], sdp[:])
                        else:
                            nc.vector.tensor_copy(ev[:], sdp[:])
                        # rows 32k+i2, cols 512q+256d+j hold
                        # sd[b, i2*128 + c*CH + 16t + 4k + 2q + d, j]
                        i0 = c * CH + 16 * t
                        sdv = sd_out[b].rearrange("(x i) j -> x i j", x=2)
                        for kk in range(4):
                            esrc = ev[32 * kk : 32 * kk + 2, :].rearrange(
                                "p (a j) -> p a j", a=4
                            )
                            edst = sdv[:, i0 + 4 * kk : i0 + 4 * kk + 4, :]
                            (nc.sync if (ev_idx + kk) % 2 else nc.scalar).dma_start(
                                edst, esrc
                            )
                        ev_idx += 1
    if split_waits:
        _split_multi_waits(nc)
    return nc


def _get_nc():
    global _CACHED_NC
    if _CACHED_NC is None:
        _CACHED_NC = _build_bass()
    return _CACHED_NC


def kernel(particles, positions, W1, b1, W2, b2):
    global LAST_RESULTS
    particles = np.ascontiguousarray(particles, dtype=np.float32)
    particlesT = np.ascontiguousarray(particles.transpose(0, 2, 1), dtype=np.float16)
    W1h = W1.astype(np.float16)
    positions = np.ascontiguousarray(positions, dtype=np.float32)
    W1 = np.ascontiguousarray(W1, dtype=np.float32)
    b1 = np.ascontiguousarray(b1, dtype=np.float32)
    W2 = np.ascontiguousarray(W2, dtype=np.float32)

    nc = _get_nc()
    in_maps = [
        {
            "particlesT": particlesT[c * BPC : (c + 1) * BPC],
            "W1": W1h,
            "b1": b1,
            "W2": W2,
        }
        for c in range(N_CORES)
    ]
    res = run_bass_kernel_spmd(nc, in_maps, core_ids=list(range(N_CORES)))
    LAST_RESULTS = res

    sd = np.concatenate([r["sd"] for r in res.results], axis=0)
    sd = sd + np.float32(b2.reshape(-1)[0])

    # loss on host in float64 (exact; trivially cheap vs the device work)
    pos64 = positions.astype(np.float64)
    diff = pos64[:, :, None, :] - pos64[:, None, :, :]
    td = np.sqrt((diff * diff).sum(-1))
    loss = np.mean((sd.astype(np.float64) - td) ** 2)

    return sd.astype(np.float32), np.float32(loss)


# revision 34
# speedup vs baseline: 1.4215x; 1.4215x over previous
"""Trainium2 Bass kernel for ContactSignedDistanceHead.

Computes, for B=16 batches of K=256 particles with D=128 features:
    hi = particles @ W1[:D]; hj = particles @ W1[D:]
    sd[b,i,j] = sum_h W2[h] * gelu(hi[b,i,h] + hj[b,j,h] + b1[h]) + b2
    loss      = mean((sd - pairwise_dist(positions))**2)

Sharding: data-parallel over batch, 2 batches per core across 8 cores.
The [B,K,K,H] intermediate (268 MB) is never materialized to HBM: each
core streams it through SBUF in [128, 32*256] chunks.

Device layout (per batch):
  - partitions p = (h, i2): p < 64 -> (h=p, i2=0), p >= 64 -> (h=p-64, i2=1)
    where i = i2*128 + i_sub.
  - hib2 [128, 128]: hib2[(h,i2), i_sub] = hi[i2*128+i_sub, h]
  - hjb2 [128, 256]: hjb2[(h,i2), j]     = hj[j, h] + b1[h]   (same for both i2)
  - X[(h,i2), (i_sub, j)] = hib2 bcast over j + hjb2 bcast over i_sub (DVE)
  - G = gelu(X)                                                 (ACT, exact)
  - sd rows: matmul with block-diag W2 lhsT [128, 2]            (PE, contract h)
    -> psum [2, N], DMA'd straight to HBM.
"""

import numpy as np

import concourse.bass as bass
import concourse.mybir as mybir
from concourse.bass_utils import run_bass_kernel_spmd
from concourse.tile import TileContext

N_CORES = 8
B, K, D, H = 16, 256, 128, 64
BPC = B // N_CORES  # batches per core

F32 = mybir.dt.float32
F16 = mybir.dt.float16
ADD = mybir.AluOpType.add

# i_sub chunk size for the streamed X/G tiles: [128, CH*256] per chunk
CH = 16
NCHUNK = 128 // CH
# which global chunk indices (b*NCHUNK+c) run their broadcast-add on GPSIMD
GP_CHUNKS = frozenset((1, 3, 5, 7, 9, 11, 13))
# fraction of evacuations on DVE: t % EVAC_MOD != 0 -> DVE
EVAC_MOD = 0

# Results of the last device run (exec_time_ns etc.), for test harnesses.
LAST_RESULTS = None
_CACHED_NC = None


def _split_multi_waits(nc):
    """The walrus codegen in this container accepts at most ONE sync-wait
    per lowered compute instruction (setupSyncWait: 'Too many sync wait
    commands'). Tile emits several. Hoist all but one wait of every such
    instruction onto standalone same-engine EventSemaphore instructions
    placed immediately before it — semantically identical (the engine
    blocks on each wait in turn instead of on all at once)."""
    skip = {"EventSemaphore", "AllEngineBarrier"}
    cnt = 0
    for fn in nc.m.functions:
        for blk in fn.blocks:
            out = []
            changed = False
            for inst in blk.instructions:
                si = inst.sync_info
                if si is not None and len(si.on_wait) > 1 and inst.opcode not in skip:
                    waits = list(si.on_wait)
                    for w in waits[:-1]:
                        cnt += 1
                        ev = mybir.InstEventSemaphore(
                            name=f"I-waitsplit-{cnt}",
                            ins=[],
                            outs=[],
                            sync_info=mybir.SyncInfo(on_wait=[w], on_update=[]),
                        )
                        ev.engine = inst.engine
                        out.append(ev)
                    inst.sync_info = mybir.SyncInfo(
                        on_wait=[waits[-1]], on_update=list(si.on_update)
                    )
                    changed = True
                out.append(inst)
            if changed:
                blk.instructions = out
    return cnt


def _build_bass(split_waits=True, psum_init=False):
    nc = bass.Bass("TRN2")

    # particles pre-transposed on host to [b, d, k] so the contract dim (d)
    # lands on partitions with a plain DMA (no PE transpose needed)
    p_in = nc.dram_tensor("particlesT", [BPC, D, K], F16, kind="ExternalInput")
    w1_in = nc.dram_tensor("W1", [2 * D, H], F16, kind="ExternalInput")
    b1_in = nc.dram_tensor("b1", [H], F32, kind="ExternalInput")
    w2_in = nc.dram_tensor("W2", [H, 1], F32, kind="ExternalInput")
    sd_out = nc.dram_tensor("sd", [BPC, K, K], F32, kind="ExternalOutput")

    gelu = mybir.ActivationFunctionType.Gelu

    with TileContext(nc) as tc:
        with (
            tc.tile_pool(name="consts", bufs=1) as cpool,
            tc.tile_pool(name="setup", bufs=2) as spool,
            tc.tile_pool(name="xg", bufs=4) as xgpool,
            tc.tile_pool(name="pss", bufs=1, space="PSUM") as pss,
            tc.tile_pool(name="psd", bufs=3, space="PSUM") as psd,
        ):
            # ---- inputs; batch-0 particles first so PE starts ASAP ----
            pT0 = spool.tile([128, K], F16, tag="pT")
            nc.sync.dma_start(pT0[:], p_in[0])
            # W1 halves as matmul stationaries [d=128, h=64]
            w1a = cpool.tile([128, H], F16)
            w1b = cpool.tile([128, H], F16)
            nc.sync.dma_start(w1a[:], w1_in[0:D, :])
            nc.sync.dma_start(w1b[:], w1_in[D:, :])

            # block-diagonal W2: rows 0:64 col 0 = W2, rows 64:128 col 1 = W2
            # fp16 so the reduce matmuls run at full PE rate (fp32 is 1/4).
            # (cast copies must be partition-aligned: engines cannot move
            # data across partitions, only DMA can.)
            w2stage = cpool.tile([128, 1], F32)
            nc.scalar.dma_start(w2stage[0:64, :], w2_in[:, :])
            nc.scalar.dma_start(w2stage[64:128, :], w2_in[:, :])
            w2blk = cpool.tile([128, 2], F16)
            nc.vector.memset(w2blk[:], 0.0)
            nc.vector.tensor_copy(w2blk[0:64, 0:1], w2stage[0:64, :])
            nc.vector.tensor_copy(w2blk[64:128, 1:2], w2stage[64:128, :])

            # b1 stacked twice along partitions -> [128, 1]
            b1t = cpool.tile([128, 1], F32)
            nc.scalar.dma_start(b1t[0:64, :], b1_in[:].unsqueeze(1))
            nc.scalar.dma_start(b1t[64:128, :], b1_in[:].unsqueeze(1))

            pT1 = spool.tile([128, K], F16, tag="pT")
            nc.sync.dma_start(pT1[:], p_in[1])
            pT_tiles = [pT0, pT1]

            ev_idx = 0
            for b in range(BPC):
                # ---- per-batch setup ----
                pT = pT_tiles[b]

                # hi and hj share one psum bank: [:, 0:128] = hi, [:, 128:384] = hj
                hij_ps = pss.tile([128, 384], F32, tag="hij", bufs=2)
                nc.tensor.matmul(
                    hij_ps[0:64, 0:128], w1a[:], pT[:, 0:128], start=True, stop=True
                )
                nc.tensor.matmul(
                    hij_ps[64:128, 0:128],
                    w1a[:],
                    pT[:, 128:256],
                    start=True,
                    stop=True,
                    tile_position=(0, 64),
                )
                nc.tensor.matmul(
                    hij_ps[0:64, 128:384], w1b[:], pT[:], start=True, stop=True
                )
                nc.tensor.matmul(
                    hij_ps[64:128, 128:384],
                    w1b[:],
                    pT[:],
                    start=True,
                    stop=True,
                    tile_position=(0, 64),
                )

                hib2 = spool.tile([128, 128], F32, tag="hib2")
                nc.vector.tensor_copy(hib2[:], hij_ps[:, 0:128])
                hjb2 = spool.tile([128, K], F32, tag="hjb2")
                nc.vector.tensor_scalar(hjb2[:], hij_ps[:, 128:384], b1t[:], None, ADD)

                # per-batch evacuation staging: chunk c's psum lands at
                # free offset c*1024; DMA'd to HBM in 4 big transfers at
                # the end of the batch
                ev_big = spool.tile([128, NCHUNK * 1024], F32, tag="evb", bufs=2)

                # ---- streamed main loop ----
                for c in range(NCHUNK):
                    x = xgpool.tile([128, CH, K], F32, tag="X")
                    addeng = nc.gpsimd if (b * NCHUNK + c) in GP_CHUNKS else nc.vector
                    addeng.tensor_tensor(
                        x[:],
                        hjb2[:].unsqueeze(1).to_broadcast((128, CH, K)),
                        hib2[:, c * CH : (c + 1) * CH]
                        .unsqueeze(2)
                        .to_broadcast((128, CH, K)),
                        ADD,
                    )
                    g = xgpool.tile([128, CH, K], F16, tag="G")
                    nc.scalar.activation(g[:], x[:], gelu)
                    gf = g[:].rearrange("p a b -> p (a b)")

                    # reduce over h via PE: block-diag W2 lhsT gives out rows
                    # (i2=0, i2=1). Four tile_position column-slots pack the
                    # outputs at partition bases 0/32/64/96 of the same psum
                    # banks, so evacuation runs at full free-dim width.
                    for t in range(CH // 16):
                        sdp = psd.tile([128, 1024], F32, tag="sd")
                        if psum_init:  # sim-only: satisfy uninit-read check
                            nc.vector.memset(sdp[:], 0.0)
                        for s in range(8):
                            kk, q = s // 2, s % 2
                            col0 = (16 * t + 2 * s) * K
                            nc.tensor.matmul(
                                sdp[32 * kk : 32 * kk + 2, 512 * q : 512 * (q + 1)],
                                w2blk[:],
                                gf[:, col0 : col0 + 512],
                                start=True,
                                stop=True,
                                tile_position=(0, 32 * kk),
                            )
                        ev = ev_big[:, (c * (CH // 16) + t) * 1024 :][:, 0:1024]
                        if EVAC_MOD and ev_idx % EVAC_MOD == 0:
                            nc.scalar.copy(ev, sdp[:])
                        else:
                            nc.vector.tensor_copy(ev, sdp[:])
                        ev_idx += 1

                # rows 32k+i2 of ev_big hold, at cols c*1024+a*256+j,
                # sd[b, i2*128 + c*16 + 4k + a, j]  (a = 2q+d)
                sdv4 = sd_out[b].rearrange("(x c g) j -> x c g j", x=2, g=16)
                for kk in range(4):
                    esrc = ev_big[32 * kk : 32 * kk + 2, :].rearrange(
                        "p (c a j) -> p c a j", c=NCHUNK, a=4
                    )
                    edst = sdv4[:, :, 4 * kk : 4 * kk + 4, :]
                    (nc.sync if kk % 2 else nc.scalar).dma_start(edst, esrc)
    if split_waits:
        _split_multi_waits(nc)
    return nc


def _get_nc():
    global _CACHED_NC
    if _CACHED_NC is None:
        _CACHED_NC = _build_bass()
    return _CACHED_NC


def kernel(particles, positions, W1, b1, W2, b2):
    global LAST_RESULTS
    particles = np.ascontiguousarray(particles, dtype=np.float32)
    particlesT = np.ascontiguousarray(particles.transpose(0, 2, 1), dtype=np.float16)
    W1h = W1.astype(np.float16)
    positions = np.ascontiguousarray(positions, dtype=np.float32)
    W1 = np.ascontiguousarray(W1, dtype=np.float32)
    b1 = np.ascontiguousarray(b1, dtype=np.float32)
    W2 = np.ascontiguousarray(W2, dtype=np.float32)

    nc = _get_nc()
    in_maps = [
        {
            "particlesT": particlesT[c * BPC : (c + 1) * BPC],
            "W1": W1h,
            "b1": b1,
            "W2": W2,
        }
        for c in range(N_CORES)
    ]
    res = run_bass_kernel_spmd(nc, in_maps, core_ids=list(range(N_CORES)))
    LAST_RESULTS = res

    sd = np.concatenate([r["sd"] for r in res.results], axis=0)
    sd = sd + np.float32(b2.reshape(-1)[0])

    # loss on host in float64 (exact; trivially cheap vs the device work)
    pos64 = positions.astype(np.float64)
    diff = pos64[:, :, None, :] - pos64[:, None, :, :]
    td = np.sqrt((diff * diff).sum(-1))
    loss = np.mean((sd.astype(np.float64) - td) ** 2)

    return sd.astype(np.float32), np.float32(loss)
